# revision 1
# baseline (speedup 1.0000x reference)
import sys

sys.path.insert(0, "/opt/trn_rl_repo")

import numpy as np

B, S, DIM, NH, HD = 2, 2048, 4096, 32, 128
NCORES = 8
HL = NH // NCORES
BS = B * S
P = 128
QT = 512
NQT = BS // QT
KO = DIM // P
SCALE = 1.0 / np.sqrt(HD)
NEG = -30000.0

_CACHE: dict = {}
DEN_BATCH = False


def _hrow(h, kind):
    half, idx = divmod(h, 2)
    return half * 768 + {"q": 0, "k": 256, "v": 512}[kind] + idx * P


def _build_nc():
    import concourse.mybir as mybir
    import concourse.tile as tile
    from concourse import bacc, bass_isa
    from concourse.masks import make_identity

    F32 = mybir.dt.float32
    F32R = mybir.dt.float32r
    AF = mybir.ActivationFunctionType
    OP = mybir.AluOpType

    nc = bacc.Bacc(trn_type="TRN2", target_bir_lowering=False, debug=False)

    xT = nc.dram_tensor("xT", [DIM, BS], F32, kind="ExternalInput").ap()
    wqkvT = nc.dram_tensor("wqkvT", [DIM, 12 * P], F32, kind="ExternalInput").ap()
    woT = nc.dram_tensor("woT", [HL * P, DIM], F32, kind="ExternalInput").ap()
    ropeA = nc.dram_tensor("ropeA", [P, BS], F32, kind="ExternalInput").ap()
    ropeB = nc.dram_tensor("ropeB", [P, BS], F32, kind="ExternalInput").ap()
    maskT = nc.dram_tensor("maskT", [QT, QT], F32, kind="ExternalInput").ap()
    outT = nc.dram_tensor("outT", [DIM, BS], F32, kind="ExternalOutput").ap()

    xT3 = xT.rearrange("(ko p) n -> p ko n", p=P)
    wqkvT3 = wqkvT.rearrange("(ko p) c -> p ko c", p=P)
    woT3 = woT.rearrange("(kc p) m -> p kc m", p=P)
    maskT3 = maskT.rearrange("(kb p) q -> p kb q", p=P)
    outT3 = outT.rearrange("(mo p) n -> p mo n", p=P)

    with tile.TileContext(nc) as tc:
        with (
            nc.allow_low_precision(reason="f32r matmul pipeline"),
            tc.tile_pool(name="const", bufs=1) as cst,
            tc.tile_pool(name="dram", bufs=1, space="DRAM") as dpool,
        ):
            qkv_d = [
                dpool.tile([P, BS], F32R, tag=f"qkvd{g}", name=f"qkvd{g}")
                for g in range(12)
            ]

            ident = cst.tile([P, P], F32)
            make_identity(nc, ident[:])

            with (
                tc.tile_pool(name="p1w", bufs=1) as wpool,
                tc.tile_pool(name="p1x", bufs=4) as xpool,
                tc.tile_pool(name="rope", bufs=1) as rpool,
                tc.tile_pool(name="p1ev", bufs=3) as evpool,
                tc.tile_pool(name="p1ps", bufs=8, space="PSUM") as pspool,
            ):
                rA = rpool.tile([P, BS], F32)
                rB = rpool.tile([P, BS], F32)

                for half in range(2):
                    w_sb = wpool.tile([P, KO, 768], F32R, tag="w")

                    def emit_slab(sl, half=half, w_sb=w_sb):
                        ksl = slice(sl * 4, (sl + 1) * 4)
                        for m in range(6):
                            wc = half * 768 + m * P
                            nc.sync.dma_start(
                                w_sb[:, ksl, m * P : (m + 1) * P],
                                wqkvT3[:, ksl, wc : wc + P].bitcast(F32R),
                            )

                    emit_slab(0)
                    emit_slab(1)
                    for qt in range(NQT):
                        cols = slice(qt * QT, (qt + 1) * QT)
                        psums = [
                            pspool.tile([P, QT], F32, tag="pp", name=f"pp{half}_{qt}_{i}")
                            for i in range(6)
                        ]
                        for kc4 in range(KO // 4):
                            x_sb = xpool.tile([P, 4, QT], F32R, tag="x")
                            nc.sync.dma_start(
                                x_sb[:], xT3[:, 4 * kc4 : 4 * kc4 + 4, cols].bitcast(F32R)
                            )
                            if qt == 0 and kc4 in (1, 2, 3, 4, 5, 6):
                                emit_slab(kc4 + 1)
                            if half == 0 and qt == 0 and kc4 in (4, 5):
                                for rch in (0, 1) if kc4 == 4 else (2, 3):
                                    rsl = slice(rch * (BS // 4), (rch + 1) * (BS // 4))
                                    nc.sync.dma_start(rA[:, rsl], ropeA[:, rsl])
                                    nc.sync.dma_start(rB[:, rsl], ropeB[:, rsl])
                            for j in range(4):
                                kc = 4 * kc4 + j
                                for m in range(6):
                                    nc.tensor.matmul(
                                        psums[m][:],
                                        w_sb[:, kc, m * P : (m + 1) * P],
                                        x_sb[:, j, :],
                                        start=(kc == 0),
                                        stop=(kc == KO - 1),
                                        skip_group_check=True,
                                    )
                        for m in range(6):
                            dst = qkv_d[half * 6 + m][:, cols]
                            if m < 4:
                                t0 = evpool.tile([P, QT], F32, tag="t0")
                                if m % 2 == 0:
                                    nc.scalar.copy(t0[:], psums[m][:])
                                else:
                                    nc.vector.tensor_copy(out=t0[:], in_=psums[m][:])
                                ev1 = evpool.tile([P, QT], F32, tag="ev1")
                                nc.vector.tensor_tensor(
                                    ev1[:], t0[:], rA[:, cols], OP.mult
                                )
                                ev2 = evpool.tile([P, QT], F32, tag="ev2")
                                nc.vector.tensor_tensor(
                                    ev2[:], t0[:], rB[:, cols], OP.mult
                                )
                                ev2s = evpool.tile([P, QT], F32, tag="ev2s")
                                nc.gpsimd.dma_start(ev2s[0:64, :], ev2[64:128, :])
                                nc.gpsimd.dma_start(ev2s[64:128, :], ev2[0:64, :])
                                out_t = evpool.tile([P, QT], F32R, tag="evo")
                                nc.vector.tensor_tensor(
                                    out_t[:], ev1[:], ev2s[:], OP.add
                                )
                                nc.sync.dma_start(dst, out_t[:])
                            else:
                                out_t = evpool.tile([P, QT], F32R, tag="evo")
                                if m % 2 == 0:
                                    nc.scalar.copy(out_t[:], psums[m][:])
                                else:
                                    nc.vector.tensor_copy(out=out_t[:], in_=psums[m][:])
                                nc.sync.dma_start(dst, out_t[:])

            with (
                tc.tile_pool(name="att", bufs=1) as attpool,
                tc.tile_pool(name="p3w", bufs=4) as wpool3,
                tc.tile_pool(name="p3ps", bufs=2, space="PSUM") as ps3,
            ):
                attnT = attpool.tile([P, HL, BS], F32R)

                with (
                    tc.tile_pool(name="bh", bufs=2) as bhpool,
                    tc.tile_pool(name="pr", bufs=8) as prpool,
                    tc.tile_pool(name="sm", bufs=3) as smpool,
                    tc.tile_pool(name="msk", bufs=1) as mpool,
                    tc.tile_pool(name="psS", bufs=4, space="PSUM") as psS,
                    tc.tile_pool(name="psO", bufs=2, space="PSUM") as psO,
                ):
                    mask_sb = mpool.tile([P, 4, QT], F32)
                    nc.sync.dma_start(mask_sb[:], maskT3)
                    att_markers = []

                    for b in range(B):
                        bcols = slice(b * S, (b + 1) * S)
                        for h in range(HL):
                            qT_sb = bhpool.tile([P, S], F32R, tag="q")
                            kT_sb = bhpool.tile([P, S], F32R, tag="k")
                            vT_sb = bhpool.tile([P, S], F32R, tag="v")
                            gq = (h // 2) * 6 + (h % 2)
                            gk = (h // 2) * 6 + 2 + (h % 2)
                            gv = (h // 2) * 6 + 4 + (h % 2)
                            for ch in range(4):
                                cs = slice(ch * (S // 4), (ch + 1) * (S // 4))
                                gcs = slice(b * S + ch * (S // 4), b * S + (ch + 1) * (S // 4))
                                mk = nc.sync.dma_start(qT_sb[:, cs], qkv_d[gq][:, gcs])
                                if h == 0 and ch == 0:
                                    att_markers.append(mk)
                                nc.sync.dma_start(kT_sb[:, cs], qkv_d[gk][:, gcs])
                                nc.sync.dma_start(vT_sb[:, cs], qkv_d[gv][:, gcs])
                            v_bl = mpool.tile([P, S // P, P], F32R, tag="vb", name=f"vb{b}_{h}")
                            for kb in range(S // P):
                                tp = psS.tile([P, P], F32, tag="sP", name=f"tp{b}_{h}_{kb}")
                                nc.tensor.transpose(
                                    tp[:],
                                    vT_sb[:, kb * P : (kb + 1) * P].bitcast(F32),
                                    ident[:],
                                )
                                nc.scalar.copy(v_bl[:, kb, :], tp[:])

                            for jq in range(S // QT):
                                qsl = qT_sb[:, jq * QT : (jq + 1) * QT]
                                nkb = (jq + 1) * (QT // P)
                                outP = psO.tile([P, QT], F32, tag="outP")
                                acc = smpool.tile([P, QT], F32, tag="acc",
                                                  name=f"acc{b}_{h}_{jq}")
                                for kb in range(nkb):
                                    sP = psS.tile([P, QT], F32, tag="sP")
                                    nc.tensor.matmul(
                                        sP[:],
                                        kT_sb[:, kb * P : (kb + 1) * P],
                                        qsl,
                                        start=True,
                                        stop=True,
                                        skip_group_check=True,
                                    )
                                    if kb >= jq * (QT // P):
                                        nc.vector.tensor_tensor(
                                            sP[:],
                                            sP[:],
                                            mask_sb[:, kb - jq * (QT // P), :],
                                            OP.add,
                                        )
                                    pr = prpool.tile([P, QT], F32R, tag="pr")
                                    nc.scalar.activation(
                                        pr[:], sP[:], AF.Exp, scale=SCALE
                                    )
                                    nc.tensor.matmul(
                                        outP[:],
                                        v_bl[:, kb, :],
                                        pr[:],
                                        start=(kb == 0),
                                        stop=(kb == nkb - 1),
                                        skip_group_check=True,
                                    )
                                    if kb == 0:
                                        nc.vector.tensor_copy(
                                            out=acc[:], in_=pr[:].bitcast(F32)
                                        )
                                    else:
                                        nc.vector.tensor_tensor(
                                            acc[:], acc[:], pr[:].bitcast(F32), OP.add
                                        )
                                den_bc = smpool.tile([P, QT], F32, tag="den",
                                                     name=f"den{b}_{h}_{jq}")
                                nc.gpsimd.partition_all_reduce(
                                    den_bc[:], acc[:], channels=P,
                                    reduce_op=bass_isa.ReduceOp.add,
                                )
                                rec = smpool.tile([P, QT], F32, tag="rec")
                                nc.vector.reciprocal(rec[:], den_bc[:])
                                nc.vector.tensor_tensor(
                                    attnT[:, h, b * S + jq * QT : b * S + (jq + 1) * QT],
                                    outP[:],
                                    rec[:],
                                    OP.mult,
                                )

                with (
                    tc.tile_pool(name="p3ev", bufs=4) as evpool3,
                ):
                    for bh3 in range(B):
                        for m in range(DIM // P):
                            woc = wpool3.tile([P, HL, P], F32R, tag="woc",
                                              name=f"woc{bh3}_{m}")
                            wdma = nc.sync.dma_start(
                                woc[:], woT3[:, :, m * P : (m + 1) * P].bitcast(F32R)
                            )
                            from concourse.tile_rust import add_dep_helper
                            add_dep_helper(
                                wdma.ins, att_markers[bh3].ins, sync=False,
                                reason="delay wo load until this batch's attention starts",
                            )
                            for qt3 in range(NQT // B):
                                qt = bh3 * (NQT // B) + qt3
                                cols = slice(qt * QT, (qt + 1) * QT)
                                oP = ps3.tile([P, QT], F32, tag="oP")
                                for kc in range(HL):
                                    nc.tensor.matmul(
                                        oP[:],
                                        woc[:, kc, :],
                                        attnT[:, kc, cols],
                                        start=(kc == 0),
                                        stop=(kc == HL - 1),
                                        skip_group_check=True,
                                    )
                                ev = evpool3.tile([P, QT], F32, tag="oev")
                                if m % 2 == 0:
                                    nc.scalar.copy(ev[:], oP[:])
                                else:
                                    nc.vector.tensor_copy(out=ev[:], in_=oP[:])
                                nc.sync.dma_start(outT[m * P : (m + 1) * P, cols], ev[:])
    nc.compile()
    return nc


def _prep_inputs(x, wq, wk, wv, wo, freqs_cos, freqs_sin, mask):
    x = np.asarray(x, dtype=np.float32)
    wq, wk, wv, wo = (np.asarray(a, dtype=np.float32) for a in (wq, wk, wv, wo))
    freqs_cos = np.asarray(freqs_cos, dtype=np.float32)
    freqs_sin = np.asarray(freqs_sin, dtype=np.float32)
    mask = np.asarray(mask, dtype=np.float32)

    xT = np.ascontiguousarray(x.reshape(BS, DIM).T)

    cosT = freqs_cos.T
    sinT = freqs_sin.T
    ropeA = np.ascontiguousarray(
        np.tile(np.concatenate([cosT, cosT], axis=0), (1, B))
    ).astype(np.float32)
    ropeB = np.ascontiguousarray(
        np.tile(np.concatenate([sinT, -sinT], axis=0), (1, B))
    ).astype(np.float32)

    band = np.maximum(mask[:QT, :QT].T, NEG).astype(np.float32)
    band = np.ascontiguousarray(band)

    perm = np.concatenate([np.arange(0, HD, 2), np.arange(1, HD, 2)])

    in_maps = []
    for c in range(NCORES):
        heads = [c * HL + j for j in range(HL)]
        cols = []
        for half in range(2):
            hA, hB = heads[2 * half], heads[2 * half + 1]
            cols.append(wq[hA * HD : (hA + 1) * HD][perm].T)
            cols.append(wq[hB * HD : (hB + 1) * HD][perm].T)
            cols.append(wk[hA * HD : (hA + 1) * HD][perm].T)
            cols.append(wk[hB * HD : (hB + 1) * HD][perm].T)
            cols.append(wv[hA * HD : (hA + 1) * HD].T)
            cols.append(wv[hB * HD : (hB + 1) * HD].T)
        wqkvT = np.ascontiguousarray(np.concatenate(cols, axis=1))
        woT = np.ascontiguousarray(wo[:, c * HL * HD : (c + 1) * HL * HD].T)
        in_maps.append(
            {
                "xT": xT,
                "wqkvT": wqkvT,
                "woT": woT,
                "ropeA": ropeA,
                "ropeB": ropeB,
                "maskT": band,
            }
        )
    return in_maps


def kernel(x, wq, wk, wv, wo, freqs_cos, freqs_sin, mask, start_pos=0):
    from concourse import bass_utils

    if "nc" not in _CACHE:
        _CACHE["nc"] = _build_nc()
    nc = _CACHE["nc"]

    in_maps = _prep_inputs(x, wq, wk, wv, wo, freqs_cos, freqs_sin, mask)
    res = bass_utils.run_bass_kernel_spmd(nc, in_maps, list(range(NCORES)))
    acc = np.zeros((DIM, BS), dtype=np.float64)
    for c in range(NCORES):
        acc += res.results[c]["outT"]
    return np.ascontiguousarray(acc.T).reshape(B, S, DIM).astype(np.float32)



# revision 8
# speedup vs baseline: 1.0239x; 1.0239x over previous
import sys

sys.path.insert(0, "/opt/trn_rl_repo")

import numpy as np

B, S, DIM, NH, HD = 2, 2048, 4096, 32, 128
NCORES = 8
HL = NH // NCORES
BS = B * S
P = 128
QT = 512
NQT = BS // QT
KO = DIM // P
SCALE = 1.0 / np.sqrt(HD)

_CACHE: dict = {}


def _build_nc():
    import concourse.mybir as mybir
    import concourse.tile as tile
    from concourse import bacc, bass_isa
    from concourse.masks import make_identity

    F32 = mybir.dt.float32
    BF16 = mybir.dt.bfloat16
    AF = mybir.ActivationFunctionType
    OP = mybir.AluOpType

    nc = bacc.Bacc(trn_type="TRN2", target_bir_lowering=False, debug=False)

    xT = nc.dram_tensor("xT", [DIM, BS], BF16, kind="ExternalInput").ap()
    wqkvT = nc.dram_tensor("wqkvT", [DIM, 12 * P], BF16, kind="ExternalInput").ap()
    woT = nc.dram_tensor("woT", [HL * P, DIM], BF16, kind="ExternalInput").ap()
    ropeA = nc.dram_tensor("ropeA", [P, BS], BF16, kind="ExternalInput").ap()
    ropeB = nc.dram_tensor("ropeB", [P, BS], BF16, kind="ExternalInput").ap()
    maskT = nc.dram_tensor("maskT", [QT, QT], BF16, kind="ExternalInput").ap()
    outT = nc.dram_tensor("outT", [DIM, BS], BF16, kind="ExternalOutput").ap()

    xT3 = xT.rearrange("(ko p) n -> p ko n", p=P)
    wqkvT3 = wqkvT.rearrange("(ko p) c -> p ko c", p=P)
    woT3 = woT.rearrange("(kc p) m -> p kc m", p=P)
    maskT3 = maskT.rearrange("(kb p) q -> p kb q", p=P)

    with tile.TileContext(nc) as tc:
        with (
            nc.allow_low_precision(reason="bf16 matmul pipeline"),
            tc.tile_pool(name="const", bufs=1) as cst,
            tc.tile_pool(name="resident", bufs=1) as res,
        ):
            identb = cst.tile([P, P], BF16)
            make_identity(nc, identb[:])

            qkv_sb = res.tile([P, 12, BS], BF16)

            with (
                tc.tile_pool(name="p1w", bufs=1) as wpool,
                tc.tile_pool(name="p1x", bufs=4) as xpool,
                tc.tile_pool(name="rope", bufs=1) as rpool,
                tc.tile_pool(name="p1ev", bufs=2) as evpool,
                tc.tile_pool(name="p1ps", bufs=8, space="PSUM") as pspool,
            ):
                rA = rpool.tile([P, BS], BF16)
                rB = rpool.tile([P, BS], BF16)

                for half in range(2):
                    w_sb = wpool.tile([P, KO, 768], BF16, tag="w")

                    def emit_slab(sl, half=half, w_sb=w_sb):
                        ksl = slice(sl * 4, (sl + 1) * 4)
                        for m in range(6):
                            wc = half * 768 + m * P
                            nc.sync.dma_start(
                                w_sb[:, ksl, m * P : (m + 1) * P],
                                wqkvT3[:, ksl, wc : wc + P],
                            )

                    emit_slab(0)
                    emit_slab(1)
                    for qt in range(NQT):
                        cols = slice(qt * QT, (qt + 1) * QT)
                        psums = [
                            pspool.tile([P, QT], F32, tag="pp", name=f"pp{half}_{qt}_{i}")
                            for i in range(6)
                        ]
                        for kc4 in range(KO // 4):
                            x_sb = xpool.tile([P, 4, QT], BF16, tag="x")
                            nc.sync.dma_start(
                                x_sb[:], xT3[:, 4 * kc4 : 4 * kc4 + 4, cols]
                            )
                            if qt == 0 and kc4 in (1, 2, 3, 4, 5, 6):
                                emit_slab(kc4 + 1)
                            if half == 0 and qt == 0 and kc4 in (4, 5):
                                for rch in (0, 1) if kc4 == 4 else (2, 3):
                                    rsl = slice(rch * (BS // 4), (rch + 1) * (BS // 4))
                                    nc.sync.dma_start(rA[:, rsl], ropeA[:, rsl])
                                    nc.sync.dma_start(rB[:, rsl], ropeB[:, rsl])
                            for j in range(4):
                                kc = 4 * kc4 + j
                                for m in range(6):
                                    nc.tensor.matmul(
                                        psums[m][:],
                                        w_sb[:, kc, m * P : (m + 1) * P],
                                        x_sb[:, j, :],
                                        start=(kc == 0),
                                        stop=(kc == KO - 1),
                                        skip_group_check=True,
                                    )
                        for m in range(6):
                            g = half * 6 + m
                            dst = qkv_sb[:, g, cols]
                            if m < 4:
                                t0 = evpool.tile([P, QT], BF16, tag="t0")
                                if m % 2 == 0:
                                    nc.scalar.copy(t0[:], psums[m][:])
                                else:
                                    nc.vector.tensor_copy(out=t0[:], in_=psums[m][:])
                                ev1 = evpool.tile([P, QT], BF16, tag="ev1")
                                nc.vector.tensor_tensor(
                                    ev1[:], t0[:], rA[:, cols], OP.mult
                                )
                                ev2 = evpool.tile([P, QT], BF16, tag="ev2")
                                nc.vector.tensor_tensor(
                                    ev2[:], t0[:], rB[:, cols], OP.mult
                                )
                                ev2s = evpool.tile([P, QT], BF16, tag="ev2s")
                                nc.gpsimd.dma_start(ev2s[0:64, :], ev2[64:128, :])
                                nc.gpsimd.dma_start(ev2s[64:128, :], ev2[0:64, :])
                                nc.vector.tensor_tensor(
                                    dst, ev1[:], ev2s[:], OP.add
                                )
                            else:
                                nc.scalar.copy(dst, psums[m][:])

            with (
                tc.tile_pool(name="p3w", bufs=4) as wpool3,
                tc.tile_pool(name="p3ps", bufs=2, space="PSUM") as ps3,
                tc.tile_pool(name="attn", bufs=1) as atpool,
            ):
                attnT = atpool.tile([P, HL, BS], BF16)
                with (
                    tc.tile_pool(name="vb", bufs=2) as vbpool,
                    tc.tile_pool(name="pr", bufs=18) as prpool,
                    tc.tile_pool(name="tree", bufs=16) as trpool,
                    tc.tile_pool(name="sm", bufs=2) as smpool,
                    tc.tile_pool(name="msk", bufs=1) as mpool,
                    tc.tile_pool(name="psS", bufs=3, space="PSUM") as psS,
                    tc.tile_pool(name="psO", bufs=2, space="PSUM") as psO,
                    tc.tile_pool(name="psT", bufs=1, space="PSUM") as psT,
                ):
                    mask_sb = mpool.tile([P, 4, QT], BF16)
                    nc.sync.dma_start(mask_sb[:], maskT3)
                    att_markers = []

                    for b in range(B):
                        bcol0 = b * S
                        for h in range(HL):
                            gq = (h // 2) * 6 + (h % 2)
                            gk = (h // 2) * 6 + 2 + (h % 2)
                            gv = (h // 2) * 6 + 4 + (h % 2)
                            v_bl = vbpool.tile([P, S // P, P], BF16, tag="vb",
                                               name=f"vb{b}_{h}")
                            for kb in range(S // P):
                                tp = psT.tile([P, P], BF16, tag="tp",
                                              name=f"tp{b}_{h}_{kb}")
                                mk = nc.tensor.transpose(
                                    tp[:],
                                    qkv_sb[:, gv, bcol0 + kb * P : bcol0 + (kb + 1) * P],
                                    identb[:],
                                )
                                if h == 0 and kb == 0:
                                    att_markers.append(mk)
                                nc.vector.tensor_copy(out=v_bl[:, kb, :], in_=tp[:])

                            for jq in range(S // QT):
                                qsl = qkv_sb[:, gq, bcol0 + jq * QT : bcol0 + (jq + 1) * QT]
                                nkb = (jq + 1) * (QT // P)
                                outP = psO.tile([P, QT], F32, tag="outP")
                                prs = []
                                pend = None
                                for kb in range(nkb):
                                    sP = psS.tile([P, QT], F32, tag="sP")
                                    nc.tensor.matmul(
                                        sP[:],
                                        qkv_sb[:, gk, bcol0 + kb * P : bcol0 + (kb + 1) * P],
                                        qsl,
                                        start=True,
                                        stop=True,
                                        skip_group_check=True,
                                    )
                                    pr = prpool.tile([P, QT], BF16, tag="pr")
                                    nc.scalar.activation(
                                        pr[:], sP[:], AF.Exp, scale=SCALE
                                    )
                                    if kb >= jq * (QT // P):
                                        nc.vector.tensor_tensor(
                                            pr[:],
                                            pr[:],
                                            mask_sb[:, kb - jq * (QT // P), :],
                                            OP.mult,
                                        )
                                    prs.append(pr)
                                    if pend is not None:
                                        nc.tensor.matmul(
                                            pend["out"], pend["st"], pend["mv"],
                                            start=pend["start"], stop=pend["stop"],
                                            skip_group_check=True,
                                        )
                                    pend = dict(
                                        out=outP[:], st=v_bl[:, kb, :], mv=pr[:],
                                        start=(kb == 0), stop=(kb == nkb - 1),
                                    )
                                    if kb == nkb - 1:
                                        nc.tensor.matmul(
                                            pend["out"], pend["st"], pend["mv"],
                                            start=pend["start"], stop=pend["stop"],
                                            skip_group_check=True,
                                        )
                                        pend = None
                                level = prs
                                while len(level) > 1:
                                    nxt = []
                                    final = len(level) == 2
                                    for i in range(0, len(level) - 1, 2):
                                        if final:
                                            t = smpool.tile([P, QT], F32, tag="acc")
                                        else:
                                            t = trpool.tile([P, QT], BF16, tag="tr")
                                        nc.vector.tensor_tensor(
                                            t[:], level[i][:], level[i + 1][:], OP.add
                                        )
                                        nxt.append(t)
                                    if len(level) % 2:
                                        nxt.append(level[-1])
                                    level = nxt
                                acc = level[0]
                                den_bc = smpool.tile([P, QT], F32, tag="den")
                                nc.gpsimd.partition_all_reduce(
                                    den_bc[:], acc[:], channels=P,
                                    reduce_op=bass_isa.ReduceOp.add,
                                )
                                rec = smpool.tile([P, QT], F32, tag="rec")
                                nc.vector.reciprocal(rec[:], den_bc[:])
                                nc.vector.tensor_tensor(
                                    attnT[:, h, bcol0 + jq * QT : bcol0 + (jq + 1) * QT],
                                    outP[:],
                                    rec[:],
                                    OP.mult,
                                )

                with (
                    tc.tile_pool(name="p3ev", bufs=4) as evpool3,
                ):
                    for bh3 in range(B):
                        for m in range(DIM // P):
                            woc = wpool3.tile([P, HL, P], BF16, tag="woc",
                                              name=f"woc{bh3}_{m}")
                            wdma = nc.sync.dma_start(
                                woc[:], woT3[:, :, m * P : (m + 1) * P]
                            )
                            from concourse.tile_rust import add_dep_helper
                            add_dep_helper(
                                wdma.ins, att_markers[bh3].ins, sync=False,
                                reason="delay wo load until this batch's attention starts",
                            )
                            for qt3 in range(NQT // B):
                                qt = bh3 * (NQT // B) + qt3
                                cols = slice(qt * QT, (qt + 1) * QT)
                                oP = ps3.tile([P, QT], F32, tag="oP")
                                for kc in range(HL):
                                    nc.tensor.matmul(
                                        oP[:],
                                        woc[:, kc, :],
                                        attnT[:, kc, cols],
                                        start=(kc == 0),
                                        stop=(kc == HL - 1),
                                        skip_group_check=True,
                                    )
                                ev = evpool3.tile([P, QT], BF16, tag="oev")
                                if m % 2 == 0:
                                    nc.scalar.copy(ev[:], oP[:])
                                else:
                                    nc.vector.tensor_copy(out=ev[:], in_=oP[:])
                                nc.sync.dma_start(outT[m * P : (m + 1) * P, cols], ev[:])
    nc.compile()
    return nc


def _prep_inputs(x, wq, wk, wv, wo, freqs_cos, freqs_sin, mask):
    import ml_dtypes

    bf16 = ml_dtypes.bfloat16
    x = np.asarray(x, dtype=np.float32)
    wq, wk, wv, wo = (np.asarray(a, dtype=np.float32) for a in (wq, wk, wv, wo))
    freqs_cos = np.asarray(freqs_cos, dtype=np.float32)
    freqs_sin = np.asarray(freqs_sin, dtype=np.float32)
    mask = np.asarray(mask, dtype=np.float32)

    xT = np.ascontiguousarray(x.reshape(BS, DIM).T.astype(bf16))

    cosT = freqs_cos.T
    sinT = freqs_sin.T
    ropeA = np.ascontiguousarray(
        np.tile(np.concatenate([cosT, cosT], axis=0), (1, B))
    ).astype(bf16)
    ropeB = np.ascontiguousarray(
        np.tile(np.concatenate([sinT, -sinT], axis=0), (1, B))
    ).astype(bf16)

    band01 = (mask[:QT, :QT].T >= 0.0).astype(bf16)
    band01 = np.ascontiguousarray(band01)

    perm = np.concatenate([np.arange(0, HD, 2), np.arange(1, HD, 2)])

    in_maps = []
    for c in range(NCORES):
        heads = [c * HL + j for j in range(HL)]
        cols = []
        for half in range(2):
            hA, hB = heads[2 * half], heads[2 * half + 1]
            cols.append(wq[hA * HD : (hA + 1) * HD][perm].T)
            cols.append(wq[hB * HD : (hB + 1) * HD][perm].T)
            cols.append(wk[hA * HD : (hA + 1) * HD][perm].T)
            cols.append(wk[hB * HD : (hB + 1) * HD][perm].T)
            cols.append(wv[hA * HD : (hA + 1) * HD].T)
            cols.append(wv[hB * HD : (hB + 1) * HD].T)
        wqkvT = np.ascontiguousarray(
            np.concatenate(cols, axis=1).astype(bf16)
        )
        woT = np.ascontiguousarray(
            wo[:, c * HL * HD : (c + 1) * HL * HD].T.astype(bf16)
        )
        in_maps.append(
            {
                "xT": xT,
                "wqkvT": wqkvT,
                "woT": woT,
                "ropeA": ropeA,
                "ropeB": ropeB,
                "maskT": band01,
            }
        )
    return in_maps


def kernel(x, wq, wk, wv, wo, freqs_cos, freqs_sin, mask, start_pos=0):
    from concourse import bass_utils

    if "nc" not in _CACHE:
        _CACHE["nc"] = _build_nc()
    nc = _CACHE["nc"]

    in_maps = _prep_inputs(x, wq, wk, wv, wo, freqs_cos, freqs_sin, mask)
    res = bass_utils.run_bass_kernel_spmd(nc, in_maps, list(range(NCORES)))
    acc = np.zeros((DIM, BS), dtype=np.float64)
    for c in range(NCORES):
        acc += res.results[c]["outT"].astype(np.float32)
    return np.ascontiguousarray(acc.T).reshape(B, S, DIM).astype(np.float32)
